# revision 10
# baseline (speedup 1.0000x reference)
"""BinaryTreeLSTM Trainium2 kernel.

Reference computation (per tree, B=64 trees, L=2048 leaves, H=DW=128):
    h = emb[word_ids] @ W_wh                       # leaves
    repeat 11x: pair adjacent siblings,
        s  = hl@W_hh + hr@W_hh + 2*b_hh            # = (hl+hr)@W_hh + 2b
        i,lf,rf,o = sigmoid(s[:4]);  g = tanh(s[4])
        c  = i*g + lf*hl + rf*hr
        h  = o*tanh(c)
    return root h                                   # [B, 1, H]

Distribution: data-parallel over trees, 8 trees per NeuronCore (x8 cores).

Per-core layout: everything transposed, hT = [H=128 partitions, nodes in the
free dim].  Gate matmuls compute s^T = W_g^T @ (hl+hr)^T with the sibling sum
folded into PSUM accumulation of two matmuls (lhsT = W_hh gate slice).
Sibling pairing is kept contiguous via a deinterleaved block layout: each
level's buffer is [L-half | R-half] with partner nodes at the same offset of
each half; outputs are written back deinterleaved by output-block parity.
Once a level fits in one 128-column block, ordering is natural and stride-2
slicing is used.

Embedding gather (the memory-bound part) is two-stage via the fast
InstDMAGatherAnt path, which needs int16 indices:
  host: per core, sort-unique the 16K token ids (<= 16384 unique, so every
        index fits in int16 after per-32768-row segmenting) and precompute
        (a) stage-1 per-segment compact-table gather lists and (b) stage-2
        per-token lists that also bake in the leaf column permutation.
  dev:  stage 1 gathers unique rows HBM->SBUF (fp32) into a compact table,
        cast to bf16; stage 2 gathers per-token rows SBUF->SBUF with
        transpose=True, directly producing X^T (feature-on-partition) in
        leaf order -- no PE transposes needed.
Fallback (index-distribution overflow, ~13-sigma event): slow per-128-row
indirect DMA gather path.
"""

import sys
from contextlib import ExitStack

import numpy as np

try:
    import concourse.bass as bass
except ImportError:  # pragma: no cover
    sys.path.insert(0, "/opt/trn_rl_repo")
    import concourse.bass as bass

import concourse.tile as tile
from concourse import bacc, mybir
from concourse.bass import IndirectOffsetOnAxis
from concourse.bass_utils import run_bass_kernel_spmd
from concourse.masks import make_identity

F32 = mybir.dt.float32
BF16 = mybir.dt.bfloat16
DT_H = BF16      # storage dtype for h buffers / gates / matmul operands
AF = mybir.ActivationFunctionType

B, L, H, DW, V = 64, 2048, 128, 128, 100000
NCORES = 8
TPC = B // NCORES          # trees per core
N0 = TPC * L               # leaf tokens per core (16384)
P = 128                    # partitions
J = N0 // P                # ids-tile columns (128)
GK = 16                    # gather columns per leaf chunk group (fallback path)
WCH = 512                  # tree-level chunk width (output cols)

SEG = 32768                # int16-addressable table segment
SEGLO = [0, SEG, 2 * SEG, 3 * SEG]
SEGHI = [SEG, 2 * SEG, 3 * SEG, V]
CAPS = [6144, 6144, 6144, 2048]          # compact slots per segment (2048-mult)
OFFS = [0, 6144, 12288, 18432]
TROWS = sum(CAPS)                         # 20480 compact-table rows
SCH = 2048                                # rows per dma_gather call
SEGMAP = [0, 0, 0, 1, 1, 1, 2, 2, 2, 3]   # stage-1 chunk -> segment


def _cdiv(a, b):
    return (a + b - 1) // b


def build_program(zero_bias: bool, repeat: int = 1, phase: str = "full",
                  use_dg: bool = True) -> bass.Bass:
    nc = bacc.Bacc(None, target_bir_lowering=False)
    ids_d = nc.dram_tensor("ids", [P, J], mybir.dt.int32, kind="ExternalInput")
    st1_d = nc.dram_tensor("st1", [P, TROWS // 16], mybir.dt.int16,
                           kind="ExternalInput")
    st2_d = nc.dram_tensor("st2", [P, N0 // 16], mybir.dt.int16,
                           kind="ExternalInput")
    emb_d = nc.dram_tensor("emb", [V, DW], F32, kind="ExternalInput")
    wwh_d = nc.dram_tensor("w_wh", [DW, H], F32, kind="ExternalInput")
    whh_d = nc.dram_tensor("w_hh", [H, 5 * H], F32, kind="ExternalInput")
    b_d = nc.dram_tensor("b_hh", [5 * H], F32, kind="ExternalInput")
    out_d = nc.dram_tensor("out", [H, TPC], F32, kind="ExternalOutput")

    with tile.TileContext(nc) as tc:
        with ExitStack() as ctx:
            st = _setup(tc, ctx, ids_d, st1_d, st2_d, wwh_d, whh_d, b_d,
                        zero_bias, use_dg)
            for _ in range(repeat):
                _emit_body(tc, emb_d, out_d, zero_bias, phase, use_dg, st)
    nc.finalize()
    return nc


def _setup(tc, ctx, ids_d, st1_d, st2_d, wwh_d, whh_d, b_d, zero_bias, use_dg):
    nc = tc.nc
    persist = ctx.enter_context(tc.tile_pool(name="persist", bufs=1))
    sb = ctx.enter_context(tc.tile_pool(name="stream", bufs=2))
    xp = ctx.enter_context(tc.tile_pool(name="xchunk", bufs=2))
    ps = ctx.enter_context(tc.tile_pool(name="psum", bufs=1, space="PSUM"))

    wwh32 = persist.tile([DW, H], F32, tag="wwh32")
    whh32 = persist.tile([H, 5 * H], F32, tag="whh32")
    wwh = persist.tile([DW, H], DT_H, tag="wwh")
    whh = persist.tile([H, 5 * H], DT_H, tag="whh")
    hA = persist.tile([P, N0], DT_H, tag="hA")
    hB = persist.tile([P, N0 // 2], DT_H, tag="hB")
    nc.sync.dma_start(out=wwh32[:], in_=wwh_d[:])
    nc.sync.dma_start(out=whh32[:], in_=whh_d[:])
    nc.vector.tensor_copy(out=wwh[:], in_=wwh32[:])
    nc.vector.tensor_copy(out=whh[:], in_=whh32[:])
    st = dict(persist=persist, sb=sb, xp=xp, ps=ps, wwh=wwh, whh=whh,
              hA=hA, hB=hB)
    if use_dg:
        idst1 = persist.tile([P, TROWS // 16], mybir.dt.int16, tag="st1")
        idst2 = persist.tile([P, N0 // 16], mybir.dt.int16, tag="st2")
        t16 = persist.tile([P, TROWS], DT_H, tag="t16")
        nc.sync.dma_start(out=idst1[:], in_=st1_d[:])
        nc.sync.dma_start(out=idst2[:], in_=st2_d[:])
        st.update(idst1=idst1, idst2=idst2, t16=t16)
    else:
        ident = persist.tile([P, P], F32, tag="ident")
        idst = persist.tile([P, J], mybir.dt.int32, tag="ids")
        nc.sync.dma_start(out=idst[:], in_=ids_d[:])
        make_identity(nc, ident[:])
        st.update(ident=ident, idst=idst)
    if not zero_bias:
        braw = persist.tile([P, 5], F32, tag="braw")
        bias2 = persist.tile([P, 5], F32, tag="bias2")
        nc.sync.dma_start(out=braw[:], in_=b_d.rearrange("(g h) -> h g", h=H))
        nc.vector.tensor_scalar_mul(bias2[:], braw[:], 2.0)
        st["bias2"] = bias2
    else:
        st["bias2"] = None
    return st


def _emit_leaf_dg(tc, emb_d, st):
    """Two-stage dma_gather leaf: compact-table gather + transpose-gather,
    then hT = W_wh^T @ X^T straight into hA (already in leaf order)."""
    nc = tc.nc
    xp, ps = st["xp"], st["ps"]
    wwh, hA = st["wwh"], st["hA"]
    idst1, idst2, t16 = st["idst1"], st["idst2"], st["t16"]

    # stage 1: HBM -> compact fp32 slabs -> bf16 compact table
    for c in range(TROWS // SCH):
        s = SEGMAP[c]
        t32 = xp.tile([P, SCH], F32, tag="t32")
        nc.gpsimd.dma_gather(
            out_ap=t32[:].rearrange("p (b d) -> p b d", d=DW),
            in_ap=emb_d[SEGLO[s] : SEGHI[s], :],
            idxs_ap=idst1[:, c * (SCH // 16) : (c + 1) * (SCH // 16)],
            num_idxs=SCH,
            num_idxs_reg=SCH,
            elem_size=DW,
            single_packet=False,
        )
        nc.vector.tensor_copy(out=t16[:, c * SCH : (c + 1) * SCH], in_=t32[:])

    # stage 2: SBUF transpose-gather (per-token, leaf-ordered) + leaf matmul
    for k in range(N0 // SCH):
        xt = xp.tile([P, SCH], DT_H, tag="xt2")
        nc.gpsimd.dma_gather(
            out_ap=xt[:].rearrange("p (one n) -> p one n", one=1),
            in_ap=t16[:].rearrange("p (t d) -> p t d", d=DW),
            idxs_ap=idst2[:, k * (SCH // 16) : (k + 1) * (SCH // 16)],
            num_idxs=SCH,
            num_idxs_reg=SCH,
            elem_size=DW,
            transpose=True,
            single_packet=False,
            sbuf_tokens_per_rank=P,
            sbuf_free_dim_per_rank=DW * 2,
        )
        for q in range(SCH // 512):
            hps = ps.tile([H, 512], F32, tag="hleaf")
            nc.tensor.matmul(
                out=hps[:], lhsT=wwh[:], rhs=xt[:, q * 512 : (q + 1) * 512],
                start=True, stop=True,
            )
            base = k * SCH + q * 512
            nc.scalar.activation(
                out=hA[:, base : base + 512], in_=hps[:], func=AF.Copy
            )


def _emit_leaf_indirect(tc, emb_d, st, phase):
    """Fallback leaf: per-128-row indirect DMA + PE transpose."""
    nc = tc.nc
    sb, xp, ps = st["sb"], st["xp"], st["ps"]
    wwh, hA = st["wwh"], st["hA"]
    ident, idst = st["ident"], st["idst"]
    half0 = N0 // 2
    for gc in range(J // GK):
        xch = xp.tile([P, GK * DW], F32, tag="x")
        for jj in range(GK):
            j = gc * GK + jj
            nc.gpsimd.indirect_dma_start(
                out=xch[:, jj * DW : (jj + 1) * DW],
                out_offset=None,
                in_=emb_d[:],
                in_offset=IndirectOffsetOnAxis(ap=idst[:, j : j + 1], axis=0),
            )
        if phase == "gather":
            continue
        for lc in range(GK // 4):
            xt = sb.tile([DW, 4 * P], DT_H, tag="xt")
            for tt in range(4):
                tr = ps.tile([DW, P], F32, tag=f"tr{tt % 2}")
                nc.tensor.transpose(
                    out=tr[:],
                    in_=xch[:, (lc * 4 + tt) * DW : (lc * 4 + tt + 1) * DW],
                    identity=ident[:],
                )
                nc.vector.tensor_copy(out=xt[:, tt * P : (tt + 1) * P], in_=tr[:])
            hps = ps.tile([H, 4 * P], F32, tag="hleaf")
            nc.tensor.matmul(out=hps[:], lhsT=wwh[:], rhs=xt[:], start=True,
                             stop=True)
            # deinterleave j-parity into [L | R] halves of hA
            j0 = gc * GK + lc * 4
            src = hps[:].rearrange("h (t two p) -> h two t p", two=2, p=P)
            for parity in range(2):
                base = parity * half0 + (j0 // 2) * P
                dst = hA[:, base : base + 2 * P].rearrange("h (a p) -> h a p", p=P)
                nc.scalar.activation(out=dst, in_=src[:, parity, :, :],
                                     func=AF.Copy)


def _emit_body(tc, emb_d, out_d, zero_bias, phase, use_dg, st):
    nc = tc.nc
    sb, ps = st["sb"], st["ps"]
    whh, hA, hB, bias2 = st["whh"], st["hA"], st["hB"], st["bias2"]

    if use_dg:
        _emit_leaf_dg(tc, emb_d, st)
    else:
        _emit_leaf_indirect(tc, emb_d, st, phase)

    if phase == "gather":
        nc.vector.memset(hA[:, 0:TPC], 0.0)
        fin32 = sb.tile([H, TPC], F32, tag="fin32")
        nc.vector.tensor_copy(out=fin32[:], in_=hA[:, 0:TPC])
        nc.sync.dma_start(out=out_d[:], in_=fin32[:])
        return

    # =========== tree phase: 11 levels ===========
    cur, cur_n = hA, N0
    level = 0
    while cur_n > TPC:
        n_out = cur_n // 2
        nxt = hB if level % 2 == 0 else hA
        halves = cur_n >= 2 * P
        for ch in range(_cdiv(n_out, WCH)):
            c0 = ch * WCH
            w = min(WCH, n_out - c0)
            if halves:
                hl = cur[:, c0 : c0 + w]
                hr = cur[:, cur_n // 2 + c0 : cur_n // 2 + c0 + w]
            else:
                pairs = cur[:, 0:cur_n].rearrange("h (x two) -> h two x", two=2)
                hl = pairs[:, 0, c0 : c0 + w]
                hr = pairs[:, 1, c0 : c0 + w]

            g4 = ps.tile([H, 4 * WCH], F32, tag="g4")
            gg = ps.tile([H, WCH], F32, tag="gg")
            for g in range(4):
                sl = slice(g * WCH, g * WCH + w)
                nc.tensor.matmul(
                    out=g4[:, sl], lhsT=whh[:, g * H : (g + 1) * H],
                    rhs=hl, start=True, stop=False,
                )
                nc.tensor.matmul(
                    out=g4[:, sl], lhsT=whh[:, g * H : (g + 1) * H],
                    rhs=hr, start=False, stop=True,
                )
            nc.tensor.matmul(
                out=gg[:, :w], lhsT=whh[:, 4 * H : 5 * H],
                rhs=hl, start=True, stop=False,
            )
            nc.tensor.matmul(
                out=gg[:, :w], lhsT=whh[:, 4 * H : 5 * H],
                rhs=hr, start=False, stop=True,
            )

            sg4 = sb.tile([H, 4 * WCH], DT_H, tag="sg4")
            tg = sb.tile([H, WCH], DT_H, tag="tg")
            if zero_bias and w == WCH:
                nc.scalar.activation(out=sg4[:], in_=g4[:], func=AF.Sigmoid)
            else:
                for g in range(4):
                    sl = slice(g * WCH, g * WCH + w)
                    kw = {} if zero_bias else {"bias": bias2[:, g : g + 1]}
                    nc.scalar.activation(
                        out=sg4[:, sl], in_=g4[:, sl], func=AF.Sigmoid, **kw
                    )
            kw = {} if zero_bias else {"bias": bias2[:, 4:5]}
            nc.scalar.activation(out=tg[:, :w], in_=gg[:, :w], func=AF.Tanh, **kw)

            i_ap = sg4[:, 0 * WCH : 0 * WCH + w]
            lf_ap = sg4[:, 1 * WCH : 1 * WCH + w]
            rf_ap = sg4[:, 2 * WCH : 2 * WCH + w]
            o_ap = sg4[:, 3 * WCH : 3 * WCH + w]

            m1 = sb.tile([H, WCH], F32, tag="m1")
            m2 = sb.tile([H, WCH], F32, tag="m2")
            m3 = sb.tile([H, WCH], F32, tag="m3")
            acc = sb.tile([H, WCH], F32, tag="acc")
            cc = sb.tile([H, WCH], F32, tag="cc")
            tcn = sb.tile([H, WCH], DT_H, tag="tcn")
            nc.vector.tensor_mul(m1[:, :w], i_ap, tg[:, :w])
            nc.gpsimd.tensor_mul(m2[:, :w], lf_ap, hl)
            nc.gpsimd.tensor_mul(m3[:, :w], rf_ap, hr)
            nc.vector.tensor_add(acc[:, :w], m1[:, :w], m2[:, :w])
            nc.vector.tensor_add(cc[:, :w], acc[:, :w], m3[:, :w])
            nc.scalar.activation(out=tcn[:, :w], in_=cc[:, :w], func=AF.Tanh)

            if n_out >= 2 * P:
                # deinterleaved write by output block parity
                t0 = c0 // (2 * P)
                tw = w // (2 * P)
                region = nxt[:, 0:n_out].rearrange("h (s t p) -> h t s p", s=2, p=P)
                dst = region[:, t0 : t0 + tw, :, :]
                o_in = o_ap.rearrange("h (t s p) -> h t s p", s=2, p=P)
                t_in = tcn[:, :w].rearrange("h (t s p) -> h t s p", s=2, p=P)
                nc.vector.tensor_mul(dst, o_in, t_in)
            else:
                nc.vector.tensor_mul(nxt[:, c0 : c0 + w], o_ap, tcn[:, :w])
        cur, cur_n = nxt, n_out
        level += 1

    fin32 = sb.tile([H, TPC], F32, tag="fin32")
    nc.vector.tensor_copy(out=fin32[:], in_=cur[:, 0:TPC])
    nc.sync.dma_start(out=out_d[:], in_=fin32[:])


_PROGRAM_CACHE: dict = {}


def _get_program(zero_bias: bool, repeat: int = 1, phase: str = "full",
                 use_dg: bool = True) -> bass.Bass:
    key = (zero_bias, repeat, phase, use_dg)
    if key not in _PROGRAM_CACHE:
        _PROGRAM_CACHE[key] = build_program(zero_bias, repeat, phase, use_dg)
    return _PROGRAM_CACHE[key]


def _wrap16(a: np.ndarray) -> np.ndarray:
    """Pack an [n] index list into the dma_gather idx layout:
    [128, n/16] int16, index i at partition i%16 column i//16, replicated to
    all eight 16-partition groups."""
    w = a.reshape(-1, 16).T.astype(np.int16)  # [16, n/16]
    return np.ascontiguousarray(np.tile(w, (8, 1)))


def _leaf_token_order() -> np.ndarray:
    """Stage-2 leaf column permutation: hT column c <- token t(c)."""
    c = np.arange(N0)
    half = N0 // 2
    left = c < half
    cc = np.where(left, c, c - half)
    u = cc // P
    p = cc % P
    j = 2 * u + (~left).astype(np.int64)
    return p * J + j  # token index


_T_OF_C = _leaf_token_order()


def _prep_core_indices(ids_c: np.ndarray):
    """Returns (st1, st2, ok) for one core's flat token ids [N0]."""
    U, inv = np.unique(ids_c, return_inverse=True)
    bounds = np.searchsorted(U, [0, SEG, 2 * SEG, 3 * SEG, V])
    counts = np.diff(bounds)
    if np.any(counts > np.array(CAPS)):
        return None, None, False
    pos = np.empty(U.size, np.int64)
    st1 = np.zeros(TROWS, np.int64)
    for s in range(4):
        lo, hi = bounds[s], bounds[s + 1]
        pos[lo:hi] = OFFS[s] + np.arange(hi - lo)
        st1[OFFS[s] : OFFS[s] + (hi - lo)] = U[lo:hi] - SEGLO[s]
    mtok = pos[inv]                      # compact position per token
    st2 = mtok[_T_OF_C]                  # leaf-ordered
    return _wrap16(st1), _wrap16(st2), True


def _make_in_maps(word_ids, emb, W_wh, W_hh, b_hh):
    word_ids = np.ascontiguousarray(np.asarray(word_ids).astype(np.int32))
    emb = np.ascontiguousarray(np.asarray(emb, dtype=np.float32))
    W_wh = np.ascontiguousarray(np.asarray(W_wh, dtype=np.float32))
    W_hh = np.ascontiguousarray(np.asarray(W_hh, dtype=np.float32))
    b_hh = np.ascontiguousarray(np.asarray(b_hh, dtype=np.float32))
    in_maps = []
    use_dg = True
    for c in range(NCORES):
        ids_c = word_ids[c * TPC : (c + 1) * TPC].reshape(-1)
        st1, st2, ok = _prep_core_indices(ids_c)
        if not ok:
            use_dg = False
        in_maps.append(
            {
                "ids": ids_c.reshape(P, J),
                "st1": st1,
                "st2": st2,
                "emb": emb,
                "w_wh": W_wh,
                "w_hh": W_hh,
                "b_hh": b_hh,
            }
        )
    if not use_dg:
        z1 = np.zeros((P, TROWS // 16), np.int16)
        z2 = np.zeros((P, N0 // 16), np.int16)
        for m in in_maps:
            m["st1"], m["st2"] = z1, z2
    return in_maps, bool(np.all(b_hh == 0.0)), use_dg


def _assemble(results) -> np.ndarray:
    out = np.empty((B, 1, H), dtype=np.float32)
    for c in range(NCORES):
        out[c * TPC : (c + 1) * TPC, 0, :] = results[c]["out"].T
    return out


def kernel(word_ids, emb, W_wh, W_hh, b_hh) -> np.ndarray:
    in_maps, zero_bias, use_dg = _make_in_maps(word_ids, emb, W_wh, W_hh, b_hh)
    nc = _get_program(zero_bias, use_dg=use_dg)
    res = run_bass_kernel_spmd(nc, in_maps, list(range(NCORES)))
    return _assemble(res.results)
